# revision 20
# baseline (speedup 1.0000x reference)
"""Trainium2 Bass kernel for DomainSpecificAttention.

Sharding: 8 cores = (2 batches) x (4 head-groups of 4 heads).  Each core
computes q/k/v projections for its 4 heads (tensor-parallel on the
projection rows), full attention for those heads, the domain-mask
reweighted attention weights (written k-major fp16, transposed + upcast
on host), and a partial out-projection summed across the 4 cores of a
batch on the host.

All heavy dataflow is in "transposed" layouts so every matmul contraction
sits on the SBUF partition axis:
  qT/kT [d, s] -> scoresT [k, q] -> exp(+ln dm[k] bias) ->
  v_aug^T @ ET = [ctx_unnorm; softmax denom; mask denom]  (one PE chain)

Precision: fp16 through the matmul dataflow (~1e-3 end to end); softmax
denominators, reciprocals and all PSUM accumulation stay fp32.

Scheduling: the two heads of a pair are interleaved k-subtile by
k-subtile so consecutive score matmuls hit alternating PE row groups
(0-63 / 64-127), letting LDWEIGHTS pull ahead and keeping the PE array
HAM-warm; each block's normalization (broadcast matmuls + DVE wmask
multiplies) is deferred by one block so the PE never idles at a block
boundary waiting on the softmax denominators.
"""
import numpy as np
from contextlib import ExitStack

import concourse.bacc as bacc
import concourse.tile as tile
from concourse import mybir
from concourse import bass_utils

B, S, E, H = 2, 2048, 1024, 16
D = E // H          # 64
NH = 4              # heads per core
KT = S // 128       # 16 k sub-tiles
F32 = mybir.dt.float32
F16 = mybir.dt.float16
AF = mybir.ActivationFunctionType

_CACHE = {}


def build_program():
    LO = F16
    LO_NP = np.float16

    nc = bacc.Bacc("TRN2", target_bir_lowering=False, debug=False, num_devices=8)
    dt_in = {}
    for name, shape, dt in [
        ("xq_t", [E, S], LO), ("xk_t", [E, S], LO), ("xv_t", [E, S], LO),
        ("wq_t", [E, 256], LO), ("wk_t", [E, 256], LO), ("wv_t", [E, 256], LO),
        ("wo_t", [256, E], LO),
        ("bq_col", [128, 2], F32), ("bk_col", [128, 2], F32),
        ("bv_row", [1, 256], LO), ("outb_row", [1, E], LO),
        ("dm_col", [128, KT], F32),
    ]:
        dt_in[name] = nc.dram_tensor(name, shape, dt, kind="ExternalInput").ap()
    part_out = nc.dram_tensor("part_out", [S, E], F32, kind="ExternalOutput").ap()
    wm_t = nc.dram_tensor("wm_t", [NH, S, S], LO, kind="ExternalOutput").ap()

    with tile.TileContext(nc) as tc, ExitStack() as top:
        cpool = top.enter_context(tc.tile_pool(name="consts", bufs=1))
        qTa = [cpool.tile([128, S], LO, name=f"qTa{p}") for p in range(2)]
        kTa = [cpool.tile([128, S], LO, name=f"kTa{p}") for p in range(2)]
        ctxTa = [cpool.tile([128, S], LO, name=f"ctxTa{p}") for p in range(2)]
        vb = cpool.tile([128, KT * NH * 104], LO, name="vb")
        wot = cpool.tile([128, 2 * E], LO)
        ones_lo = cpool.tile([1, 128], LO)
        dmc = cpool.tile([128, KT], F32)
        ln_dm = cpool.tile([128, KT], F32)
        rdm = cpool.tile([128, KT], F32)
        bqc = cpool.tile([128, 2], F32)
        bkc = cpool.tile([128, 2], F32)
        bvr = cpool.tile([1, 256], LO)
        obr = cpool.tile([1, E], LO)

        # ---- stage A: constants ----
        nc.gpsimd.dma_start(dmc[:], dt_in["dm_col"][:])
        nc.gpsimd.dma_start(bqc[:], dt_in["bq_col"][:])
        nc.gpsimd.dma_start(bkc[:], dt_in["bk_col"][:])
        nc.gpsimd.dma_start(bvr[:], dt_in["bv_row"][:])
        nc.gpsimd.dma_start(obr[:], dt_in["outb_row"][:])
        nc.gpsimd.dma_start(
            wot[:].rearrange("p (t d) -> p t d", t=2),
            dt_in["wo_t"].rearrange("(t p) d -> p t d", p=128))
        nc.scalar.activation(ln_dm[:], dmc[:], AF.Ln)
        nc.vector.reciprocal_approx_fast(rdm[:], dmc[:])
        nc.vector.tensor_scalar(ones_lo[:], obr[0:1, 0:128], 0.0, 1.0,
                                mybir.AluOpType.mult, mybir.AluOpType.add)

        # ---- stage B: projections ----
        with ExitStack() as st:
            wpool = st.enter_context(tc.tile_pool(name="wproj", bufs=1))
            xpool = st.enter_context(tc.tile_pool(name="xstream", bufs=3))
            pspool = st.enter_context(tc.tile_pool(name="psproj", bufs=2, space="PSUM"))
            wqt = wpool.tile([128, 8 * 256], LO)
            wkt = wpool.tile([128, 8 * 256], LO)
            wvt = wpool.tile([128, 8 * 256], LO)
            for wt, wname in [(wqt, "wq_t"), (wkt, "wk_t"), (wvt, "wv_t")]:
                nc.gpsimd.dma_start(
                    wt[:].rearrange("p (e d) -> p e d", e=8),
                    dt_in[wname].rearrange("(e p) d -> p e d", p=128))

            # q / k projections -> qTa/kTa (transposed, bias per-partition)
            for xname, wt, dst, bias in [("xq_t", wqt, qTa, bqc),
                                         ("xk_t", wkt, kTa, bkc)]:
                for sc in range(4):
                    xc = xpool.tile([128, 8 * 512], LO, tag="xc")
                    nc.gpsimd.dma_start(
                        xc[:].rearrange("p (e s) -> p e s", e=8),
                        dt_in[xname][:, 512 * sc:512 * (sc + 1)]
                        .rearrange("(e p) s -> p e s", p=128))
                    for p in range(2):
                        ps = pspool.tile([128, 512], F32, tag="psqk")
                        for e in range(8):
                            nc.tensor.matmul(
                                ps[:], wt[:, 256 * e + 128 * p:256 * e + 128 * (p + 1)],
                                xc[:, 512 * e:512 * (e + 1)],
                                start=(e == 0), stop=(e == 7))
                        nc.vector.tensor_scalar_add(
                            dst[p][:, 512 * sc:512 * (sc + 1)], ps[:], bias[:, p:p + 1])

            # v projection -> vb (natural layout, /dm, aug cols at 64 and 96)
            for sc in range(4):
                xcv = xpool.tile([128, 8 * 512], LO, tag="xc")
                nc.gpsimd.dma_start(
                    xcv[:].rearrange("p (e s) -> p e s", e=8),
                    dt_in["xv_t"][:, 512 * sc:512 * (sc + 1)]
                    .rearrange("(e p) s -> p e s", p=128))
                for st_ in range(4):
                    stg = 4 * sc + st_
                    psv = pspool.tile([128, 256], F32, tag="psv")
                    for e in range(8):
                        nc.tensor.matmul(
                            psv[:], xcv[:, 512 * e + 128 * st_:512 * e + 128 * (st_ + 1)],
                            wvt[:, 256 * e:256 * (e + 1)],
                            start=(e == 0), stop=False)
                    nc.tensor.matmul(psv[:], ones_lo[0:1, 0:128], bvr[0:1, :],
                                     start=False, stop=True)
                    base = 104 * NH * stg
                    nc.vector.tensor_scalar_mul(
                        vb[:, base:base + 104 * NH]
                        .rearrange("p (h c) -> p h c", h=NH)[:, :, 0:64],
                        psv[:].rearrange("p (h c) -> p h c", h=NH),
                        rdm[:, stg:stg + 1])
                    nc.vector.tensor_scalar(
                        vb[:, base:base + 104 * NH]
                        .rearrange("p (h c) -> p h c", h=NH)[:, :, 64:97],
                        psv[:].rearrange("p (h c) -> p h c", h=NH)[:, :, 0:33],
                        0.0, 1.0, mybir.AluOpType.mult, mybir.AluOpType.add)
                    for h in range(NH):
                        nc.vector.tensor_copy(
                            vb[:, base + 104 * h + 64:base + 104 * h + 65],
                            rdm[:, stg:stg + 1])

        # ---- stage C: attention (head pairs interleaved; deferred finish) ----
        with ExitStack() as st:
            etpool = st.enter_context(tc.tile_pool(name="et", bufs=2))
            wmpool = st.enter_context(tc.tile_pool(name="wmst", bufs=2))
            smpool = st.enter_context(tc.tile_pool(name="small", bufs=1))
            opool = st.enter_context(tc.tile_pool(name="oev", bufs=2))
            sm2pool = st.enter_context(tc.tile_pool(name="small2", bufs=2))
            pssc = st.enter_context(tc.tile_pool(name="pssc", bufs=2, space="PSUM"))
            psctx = st.enter_context(tc.tile_pool(name="psctx", bufs=1, space="PSUM"))
            psb = st.enter_context(tc.tile_pool(name="psb", bufs=2, space="PSUM"))

            def finish(state):
                p, q0, ETp, hstate = state
                for h2 in range(2):
                    h = 2 * p + h2
                    rows = slice(64 * h2, 64 * (h2 + 1))
                    inv16, cu = hstate[h2]
                    pbd = psb.tile([64, 512], F32, tag="bcast", name="pbd")
                    nc.tensor.matmul(pbd[:], ones_lo[0:1, 0:64],
                                     inv16[0:1, 0:512], start=True, stop=True)
                    nc.vector.tensor_mul(ctxTa[p][rows, q0:q0 + 512], cu[:], pbd[:])
                    pbw = psb.tile([128, 512], F32, tag="bcast", name="pbw")
                    nc.tensor.matmul(pbw[:], ones_lo[0:1, 0:128],
                                     inv16[0:1, 512:1024], start=True, stop=True)
                    pbw_s = sm2pool.tile([128, 4 * 512], LO, tag=f"pbw_s{h2}")
                    nc.vector.tensor_copy(pbw_s[:, 0:512], pbw[:])
                    for r in range(1, 4):
                        nc.vector.tensor_copy(pbw_s[:, 512 * r:512 * (r + 1)],
                                              pbw_s[:, 0:512])
                    for kt4 in range(4):
                        wst = wmpool.tile([128, 4 * 512], LO, tag=f"wst{h2}")
                        nc.vector.tensor_mul(
                            wst[:].rearrange("p (a q) -> p a q", a=4),
                            ETp[:, 4096 * kt4:4096 * (kt4 + 1)]
                            .rearrange("p (a q) -> p a q", q=1024)
                            [:, :, 512 * h2:512 * (h2 + 1)],
                            pbw_s[:].rearrange("p (a q) -> p a q", a=4))
                        nc.gpsimd.dma_start(
                            wm_t[h, 512 * kt4:512 * (kt4 + 1), q0:q0 + 512]
                            .rearrange("(a p) q -> p a q", p=128),
                            wst[:].rearrange("p (a q) -> p a q", a=4))

            def outproj_st(st_):
                if True:
                    ev = opool.tile([128, 1024], F32, tag="ev")
                    for j in range(2):
                        pso = psb.tile([128, 512], F32, tag="bcast", name="pso")
                        for t in range(2):
                            nc.tensor.matmul(
                                pso[:],
                                ctxTa[t][:, 128 * st_:128 * (st_ + 1)],
                                wot[:, E * t + 512 * j:E * t + 512 * (j + 1)],
                                start=(t == 0), stop=False)
                        nc.tensor.matmul(
                            pso[:], ones_lo[0:1, 0:128],
                            obr[0:1, 512 * j:512 * (j + 1)],
                            start=False, stop=True)
                        nc.scalar.activation(ev[:, 512 * j:512 * (j + 1)], pso[:], AF.Copy)
                    nc.gpsimd.dma_start(part_out[128 * st_:128 * (st_ + 1), :], ev[:])

            pending = None
            for qc in range(4):
                for p in range(2):
                    q0 = 512 * qc
                    ETp = etpool.tile([128, KT * 1024], LO, tag="ET")
                    pcx = [psctx.tile([97, 512], F32, tag=f"pcx{h2}", name=f"pcx{h2}")
                           for h2 in range(2)]
                    for kt in range(KT):
                        pss = pssc.tile([128, 1024], F32, tag="pss")
                        for h2 in range(2):
                            rows = slice(64 * h2, 64 * (h2 + 1))
                            nc.tensor.matmul(
                                pss[:, 512 * h2:512 * (h2 + 1)],
                                kTa[p][rows, 128 * kt:128 * (kt + 1)],
                                qTa[p][rows, q0:q0 + 512],
                                start=True, stop=True)
                        nc.scalar.activation(
                            ETp[:, 1024 * kt:1024 * (kt + 1)], pss[:], AF.Exp,
                            bias=ln_dm[:, kt:kt + 1])
                        for h2 in range(2):
                            nc.tensor.matmul(
                                pcx[h2][:],
                                vb[:, 104 * (NH * kt + 2 * p + h2):104 * (NH * kt + 2 * p + h2) + 97],
                                ETp[:, 1024 * kt + 512 * h2:1024 * kt + 512 * (h2 + 1)],
                                start=(kt == 0), stop=(kt == KT - 1))
                        if kt == 1 and pending is not None:
                            # previous block's normalization: PE broadcast mms hit
                            # ready inputs (no stall); DVE finish ops queue ahead
                            # of this block's own denominator work
                            finish(pending)
                        if (kt >= 8 and kt % 2 == 0 and pending is not None
                                and pending[0] == 1 and p == 0):
                            # out-projection of the completed qc group doubles as
                            # PE filler while ACT drains this block's exps
                            outproj_st(4 * (pending[1] // 512) + (kt - 8) // 2)
                    hstate = []
                    for h2 in range(2):
                        drow = smpool.tile([1, 1024], F32, tag=f"drow{h2}")
                        inv = smpool.tile([1, 1024], F32, tag=f"inv{h2}")
                        inv16 = sm2pool.tile([1, 1024], LO, tag=f"inv16{h2}")
                        cu = sm2pool.tile([64, 512], F32, tag=f"cu{h2}")
                        nc.vector.tensor_copy(drow[0:1, 0:512], pcx[h2][64:65, :])
                        nc.vector.tensor_copy(drow[0:1, 512:1024], pcx[h2][96:97, :])
                        nc.vector.reciprocal_approx_fast(inv[:], drow[:])
                        nc.vector.tensor_copy(inv16[:], inv[:])
                        nc.vector.tensor_copy(cu[:], pcx[h2][0:64, :])
                        hstate.append((inv16, cu))
                    pending = (p, q0, ETp, hstate)
            finish(pending)
            for st_ in range(12, 16):
                outproj_st(st_)

    nc.compile()
    return nc, LO_NP


def _prep_core_inputs(c, query, key, value, domain_mask, wq, wk, wv, bq, bk, bv,
                      out_w, out_b, lo=np.float16):
    b, g = c // 4, c % 4
    hs = slice(256 * g, 256 * (g + 1))
    scale = np.float32(1.0 / np.sqrt(D))
    zeros_ob = np.zeros_like(out_b)
    return {
        "xq_t": np.ascontiguousarray(query[b].T).astype(lo),
        "xk_t": np.ascontiguousarray(key[b].T).astype(lo),
        "xv_t": np.ascontiguousarray(value[b].T).astype(lo),
        "wq_t": np.ascontiguousarray((wq[hs] * scale).T).astype(lo),
        "wk_t": np.ascontiguousarray(wk[hs].T).astype(lo),
        "wv_t": np.ascontiguousarray(wv[hs].T).astype(lo),
        "wo_t": np.ascontiguousarray(out_w[:, hs].T).astype(lo),
        "bq_col": np.ascontiguousarray((bq[hs] * scale).reshape(2, 128).T),
        "bk_col": np.ascontiguousarray(bk[hs].reshape(2, 128).T),
        "bv_row": bv[hs].reshape(1, 256).astype(lo),
        "outb_row": (out_b if g == 0 else zeros_ob).reshape(1, E).astype(lo),
        "dm_col": np.ascontiguousarray(domain_mask[b].reshape(KT, 128).T),
    }


def kernel(query, key, value, domain_mask, in_proj_w, in_proj_b, out_w, out_b,
           _trace=False):
    query = np.asarray(query, np.float32)
    key = np.asarray(key, np.float32)
    value = np.asarray(value, np.float32)
    domain_mask = np.asarray(domain_mask, np.float32)
    in_proj_w = np.asarray(in_proj_w, np.float32)
    in_proj_b = np.asarray(in_proj_b, np.float32)
    out_w = np.asarray(out_w, np.float32)
    out_b = np.asarray(out_b, np.float32)

    wq, wk, wv = np.split(in_proj_w, 3, axis=0)
    bq, bk, bv = np.split(in_proj_b, 3, axis=0)

    if "nc" not in _CACHE:
        _CACHE["nc"], _CACHE["lo"] = build_program()
    nc, lo = _CACHE["nc"], _CACHE["lo"]

    in_maps = [
        _prep_core_inputs(c, query, key, value, domain_mask, wq, wk, wv,
                          bq, bk, bv, out_w, out_b, lo)
        for c in range(8)
    ]
    res = bass_utils.run_bass_kernel_spmd(nc, in_maps, core_ids=list(range(8)),
                                          trace=_trace)
    _CACHE["last_exec_time_ns"] = res.exec_time_ns

    attn_output = np.zeros((B, S, E), np.float32)
    for b in range(B):
        for c in range(4 * b, 4 * b + 4):
            attn_output[b] += res.results[c]["part_out"]
    wmask = np.empty((B, H, S, S), np.float32)
    for b in range(B):
        for hg in range(H):
            wmask[b, hg] = res.results[4 * b + hg // 4]["wm_t"][hg % 4].T.astype(np.float32)
    return attn_output, wmask


# revision 21
# speedup vs baseline: 1.1959x; 1.1959x over previous
"""Trainium2 Bass kernel for DomainSpecificAttention.

Sharding: 8 cores = (2 batches) x (4 head-groups of 4 heads).  Each core
computes q/k/v projections for its 4 heads (tensor-parallel on the
projection rows), full attention for those heads, the domain-mask
reweighted attention weights (written k-major fp16, transposed + upcast
on host), and a partial out-projection summed across the 4 cores of a
batch on the host.

All heavy dataflow is in "transposed" layouts so every matmul contraction
sits on the SBUF partition axis:
  qT/kT [d, s] -> scoresT [k, q] -> exp(+ln dm[k] bias) ->
  v_aug^T @ ET = [ctx_unnorm; softmax denom; mask denom]  (one PE chain)

Precision: fp16 through the matmul dataflow (~1e-3 end to end); softmax
denominators, reciprocals and all PSUM accumulation stay fp32.

Scheduling: the two heads of a pair are interleaved k-subtile by
k-subtile so consecutive score matmuls hit alternating PE row groups
(0-63 / 64-127), letting LDWEIGHTS pull ahead and keeping the PE array
HAM-warm; each block's normalization (broadcast matmuls + DVE wmask
multiplies) is deferred by one block so the PE never idles at a block
boundary waiting on the softmax denominators.
"""
import numpy as np
from contextlib import ExitStack

import concourse.bacc as bacc
import concourse.tile as tile
from concourse import mybir
from concourse import bass_utils

B, S, E, H = 2, 2048, 1024, 16
D = E // H          # 64
NH = 4              # heads per core
KT = S // 128       # 16 k sub-tiles
F32 = mybir.dt.float32
F16 = mybir.dt.float16
AF = mybir.ActivationFunctionType

_CACHE = {}


def build_program():
    LO = F16
    LO_NP = np.float16

    nc = bacc.Bacc("TRN2", target_bir_lowering=False, debug=False, num_devices=8)
    dt_in = {}
    for name, shape, dt in [
        ("xq_t", [E, S], LO), ("xk_t", [E, S], LO), ("xv_t", [E, S], LO),
        ("wq_t", [E, 256], LO), ("wk_t", [E, 256], LO), ("wv_t", [E, 256], LO),
        ("wo_t", [256, E], LO),
        ("bq_col", [128, 2], F32), ("bk_col", [128, 2], F32),
        ("bv_row", [1, 256], LO), ("outb_row", [1, E], LO),
        ("dm_col", [128, KT], F32),
    ]:
        dt_in[name] = nc.dram_tensor(name, shape, dt, kind="ExternalInput").ap()
    part_out = nc.dram_tensor("part_out", [S, E], F32, kind="ExternalOutput").ap()
    wm_t = nc.dram_tensor("wm_t", [NH, S, S], LO, kind="ExternalOutput").ap()

    with tile.TileContext(nc) as tc, ExitStack() as top:
        cpool = top.enter_context(tc.tile_pool(name="consts", bufs=1))
        qTa = [cpool.tile([128, S], LO, name=f"qTa{p}") for p in range(2)]
        kTa = [cpool.tile([128, S], LO, name=f"kTa{p}") for p in range(2)]
        ctxTa = [cpool.tile([128, S], LO, name=f"ctxTa{p}") for p in range(2)]
        vb = cpool.tile([128, KT * NH * 104], LO, name="vb")
        wot = cpool.tile([128, 2 * E], LO)
        ones_lo = cpool.tile([1, 128], LO)
        dmc = cpool.tile([128, KT], F32)
        ln_dm = cpool.tile([128, KT], F32)
        rdm = cpool.tile([128, KT], F32)
        bqc = cpool.tile([128, 2], F32)
        bkc = cpool.tile([128, 2], F32)
        bvr = cpool.tile([1, 256], LO)
        obr = cpool.tile([1, E], LO)

        # ---- stage A: constants ----
        nc.gpsimd.dma_start(dmc[:], dt_in["dm_col"][:])
        nc.gpsimd.dma_start(bqc[:], dt_in["bq_col"][:])
        nc.gpsimd.dma_start(bkc[:], dt_in["bk_col"][:])
        nc.gpsimd.dma_start(bvr[:], dt_in["bv_row"][:])
        nc.gpsimd.dma_start(obr[:], dt_in["outb_row"][:])
        nc.gpsimd.dma_start(
            wot[:].rearrange("p (t d) -> p t d", t=2),
            dt_in["wo_t"].rearrange("(t p) d -> p t d", p=128))
        nc.scalar.activation(ln_dm[:], dmc[:], AF.Ln)
        nc.vector.reciprocal_approx_fast(rdm[:], dmc[:])
        nc.vector.tensor_scalar(ones_lo[:], obr[0:1, 0:128], 0.0, 1.0,
                                mybir.AluOpType.mult, mybir.AluOpType.add)

        # ---- stage B: projections ----
        with ExitStack() as st:
            wpool = st.enter_context(tc.tile_pool(name="wproj", bufs=1))
            xpool = st.enter_context(tc.tile_pool(name="xstream", bufs=3))
            pspool = st.enter_context(tc.tile_pool(name="psproj", bufs=2, space="PSUM"))
            wqt = wpool.tile([128, 8 * 256], LO)
            wkt = wpool.tile([128, 8 * 256], LO)
            wvt = wpool.tile([128, 8 * 256], LO)
            for wt, wname in [(wqt, "wq_t"), (wkt, "wk_t"), (wvt, "wv_t")]:
                nc.gpsimd.dma_start(
                    wt[:].rearrange("p (e d) -> p e d", e=8),
                    dt_in[wname].rearrange("(e p) d -> p e d", p=128))

            # q / k projections -> qTa/kTa (transposed, bias per-partition)
            for xname, wt, dst, bias in [("xq_t", wqt, qTa, bqc),
                                         ("xk_t", wkt, kTa, bkc)]:
                for sc in range(4):
                    xc = xpool.tile([128, 8 * 512], LO, tag="xc")
                    nc.gpsimd.dma_start(
                        xc[:].rearrange("p (e s) -> p e s", e=8),
                        dt_in[xname][:, 512 * sc:512 * (sc + 1)]
                        .rearrange("(e p) s -> p e s", p=128))
                    for p in range(2):
                        ps = pspool.tile([128, 512], F32, tag="psqk")
                        for e in range(8):
                            nc.tensor.matmul(
                                ps[:], wt[:, 256 * e + 128 * p:256 * e + 128 * (p + 1)],
                                xc[:, 512 * e:512 * (e + 1)],
                                start=(e == 0), stop=(e == 7))
                        nc.vector.tensor_scalar_add(
                            dst[p][:, 512 * sc:512 * (sc + 1)], ps[:], bias[:, p:p + 1])

            # v projection -> vb (natural layout, /dm, aug cols at 64 and 96)
            for sc in range(4):
                xcv = xpool.tile([128, 8 * 512], LO, tag="xc")
                nc.gpsimd.dma_start(
                    xcv[:].rearrange("p (e s) -> p e s", e=8),
                    dt_in["xv_t"][:, 512 * sc:512 * (sc + 1)]
                    .rearrange("(e p) s -> p e s", p=128))
                for st_ in range(4):
                    stg = 4 * sc + st_
                    psv = pspool.tile([128, 256], F32, tag="psv")
                    for e in range(8):
                        nc.tensor.matmul(
                            psv[:], xcv[:, 512 * e + 128 * st_:512 * e + 128 * (st_ + 1)],
                            wvt[:, 256 * e:256 * (e + 1)],
                            start=(e == 0), stop=False)
                    nc.tensor.matmul(psv[:], ones_lo[0:1, 0:128], bvr[0:1, :],
                                     start=False, stop=True)
                    base = 104 * NH * stg
                    nc.vector.tensor_scalar_mul(
                        vb[:, base:base + 104 * NH]
                        .rearrange("p (h c) -> p h c", h=NH)[:, :, 0:64],
                        psv[:].rearrange("p (h c) -> p h c", h=NH),
                        rdm[:, stg:stg + 1])
                    nc.vector.tensor_scalar(
                        vb[:, base:base + 104 * NH]
                        .rearrange("p (h c) -> p h c", h=NH)[:, :, 64:97],
                        psv[:].rearrange("p (h c) -> p h c", h=NH)[:, :, 0:33],
                        0.0, 1.0, mybir.AluOpType.mult, mybir.AluOpType.add)
                    for h in range(NH):
                        nc.vector.tensor_copy(
                            vb[:, base + 104 * h + 64:base + 104 * h + 65],
                            rdm[:, stg:stg + 1])

        # ---- stage C: attention (head pairs interleaved; deferred finish) ----
        with ExitStack() as st:
            etpool = st.enter_context(tc.tile_pool(name="et", bufs=2))
            wmpool = st.enter_context(tc.tile_pool(name="wmst", bufs=2))
            smpool = st.enter_context(tc.tile_pool(name="small", bufs=1))
            opool = st.enter_context(tc.tile_pool(name="oev", bufs=2))
            sm2pool = st.enter_context(tc.tile_pool(name="small2", bufs=2))
            pssc = st.enter_context(tc.tile_pool(name="pssc", bufs=2, space="PSUM"))
            psctx = st.enter_context(tc.tile_pool(name="psctx", bufs=1, space="PSUM"))
            psb = st.enter_context(tc.tile_pool(name="psb", bufs=2, space="PSUM"))

            def finish(state):
                p, q0, ETp, hstate = state
                for h2 in range(2):
                    h = 2 * p + h2
                    rows = slice(64 * h2, 64 * (h2 + 1))
                    inv16, cu = hstate[h2]
                    pbd = psb.tile([64, 512], F32, tag="bcast", name="pbd")
                    nc.tensor.matmul(pbd[:], ones_lo[0:1, 0:64],
                                     inv16[0:1, 0:512], start=True, stop=True)
                    nc.vector.tensor_mul(ctxTa[p][rows, q0:q0 + 512], cu[:], pbd[:])
                    pbw = psb.tile([128, 512], F32, tag="bcast", name="pbw")
                    nc.tensor.matmul(pbw[:], ones_lo[0:1, 0:128],
                                     inv16[0:1, 512:1024], start=True, stop=True)
                    pbw_s = sm2pool.tile([128, 4 * 512], LO, tag=f"pbw_s{h2}")
                    nc.vector.tensor_copy(pbw_s[:, 0:512], pbw[:])
                    for r in range(1, 4):
                        nc.vector.tensor_copy(pbw_s[:, 512 * r:512 * (r + 1)],
                                              pbw_s[:, 0:512])
                    for kt4 in range(4):
                        wst = wmpool.tile([128, 4 * 512], LO, tag=f"wst{h2}")
                        nc.vector.tensor_mul(
                            wst[:].rearrange("p (a q) -> p a q", a=4),
                            ETp[:, 4096 * kt4:4096 * (kt4 + 1)]
                            .rearrange("p (a q) -> p a q", q=1024)
                            [:, :, 512 * h2:512 * (h2 + 1)],
                            pbw_s[:].rearrange("p (a q) -> p a q", a=4))
                        nc.gpsimd.dma_start(
                            wm_t[h, 512 * kt4:512 * (kt4 + 1), q0:q0 + 512]
                            .rearrange("(a p) q -> p a q", p=128),
                            wst[:].rearrange("p (a q) -> p a q", a=4))

            def outproj_st(st_):
                if True:
                    ev = opool.tile([128, 1024], F32, tag="ev")
                    for j in range(2):
                        pso = psb.tile([128, 512], F32, tag="bcast", name="pso")
                        for t in range(2):
                            nc.tensor.matmul(
                                pso[:],
                                ctxTa[t][:, 128 * st_:128 * (st_ + 1)],
                                wot[:, E * t + 512 * j:E * t + 512 * (j + 1)],
                                start=(t == 0), stop=False)
                        nc.tensor.matmul(
                            pso[:], ones_lo[0:1, 0:128],
                            obr[0:1, 512 * j:512 * (j + 1)],
                            start=False, stop=True)
                        nc.scalar.activation(ev[:, 512 * j:512 * (j + 1)], pso[:], AF.Copy)
                    nc.gpsimd.dma_start(part_out[128 * st_:128 * (st_ + 1), :], ev[:])

            pending = None
            for qc in range(4):
                for p in range(2):
                    q0 = 512 * qc
                    ETp = etpool.tile([128, KT * 1024], LO, tag="ET")
                    pcx = [psctx.tile([97, 512], F32, tag=f"pcx{h2}", name=f"pcx{h2}")
                           for h2 in range(2)]
                    for kt in range(KT):
                        pss = pssc.tile([128, 1024], F32, tag="pss")
                        for h2 in range(2):
                            rows = slice(64 * h2, 64 * (h2 + 1))
                            nc.tensor.matmul(
                                pss[:, 512 * h2:512 * (h2 + 1)],
                                kTa[p][rows, 128 * kt:128 * (kt + 1)],
                                qTa[p][rows, q0:q0 + 512],
                                start=True, stop=True)
                        nc.scalar.activation(
                            ETp[:, 1024 * kt:1024 * (kt + 1)], pss[:], AF.Exp,
                            bias=ln_dm[:, kt:kt + 1])
                        for h2 in range(2):
                            nc.tensor.matmul(
                                pcx[h2][:],
                                vb[:, 104 * (NH * kt + 2 * p + h2):104 * (NH * kt + 2 * p + h2) + 97],
                                ETp[:, 1024 * kt + 512 * h2:1024 * kt + 512 * (h2 + 1)],
                                start=(kt == 0), stop=(kt == KT - 1))
                        if kt == 1 and pending is not None:
                            # previous block's normalization: PE broadcast mms hit
                            # ready inputs (no stall); DVE finish ops queue ahead
                            # of this block's own denominator work
                            finish(pending)
                    hstate = []
                    for h2 in range(2):
                        drow = smpool.tile([1, 1024], F32, tag=f"drow{h2}")
                        inv = smpool.tile([1, 1024], F32, tag=f"inv{h2}")
                        inv16 = sm2pool.tile([1, 1024], LO, tag=f"inv16{h2}")
                        cu = sm2pool.tile([64, 512], F32, tag=f"cu{h2}")
                        nc.vector.tensor_copy(drow[0:1, 0:512], pcx[h2][64:65, :])
                        nc.vector.tensor_copy(drow[0:1, 512:1024], pcx[h2][96:97, :])
                        nc.vector.reciprocal_approx_fast(inv[:], drow[:])
                        nc.vector.tensor_copy(inv16[:], inv[:])
                        nc.vector.tensor_copy(cu[:], pcx[h2][0:64, :])
                        hstate.append((inv16, cu))
                    if pending is not None and pending[0] == 1:
                        for st_ in range(4 * (pending[1] // 512), 4 * (pending[1] // 512) + 4):
                            outproj_st(st_)
                    pending = (p, q0, ETp, hstate)
            finish(pending)
            for st_ in range(12, 16):
                outproj_st(st_)

    nc.compile()
    return nc, LO_NP


def _prep_core_inputs(c, query, key, value, domain_mask, wq, wk, wv, bq, bk, bv,
                      out_w, out_b, lo=np.float16):
    b, g = c // 4, c % 4
    hs = slice(256 * g, 256 * (g + 1))
    scale = np.float32(1.0 / np.sqrt(D))
    zeros_ob = np.zeros_like(out_b)
    return {
        "xq_t": np.ascontiguousarray(query[b].T).astype(lo),
        "xk_t": np.ascontiguousarray(key[b].T).astype(lo),
        "xv_t": np.ascontiguousarray(value[b].T).astype(lo),
        "wq_t": np.ascontiguousarray((wq[hs] * scale).T).astype(lo),
        "wk_t": np.ascontiguousarray(wk[hs].T).astype(lo),
        "wv_t": np.ascontiguousarray(wv[hs].T).astype(lo),
        "wo_t": np.ascontiguousarray(out_w[:, hs].T).astype(lo),
        "bq_col": np.ascontiguousarray((bq[hs] * scale).reshape(2, 128).T),
        "bk_col": np.ascontiguousarray(bk[hs].reshape(2, 128).T),
        "bv_row": bv[hs].reshape(1, 256).astype(lo),
        "outb_row": (out_b if g == 0 else zeros_ob).reshape(1, E).astype(lo),
        "dm_col": np.ascontiguousarray(domain_mask[b].reshape(KT, 128).T),
    }


def kernel(query, key, value, domain_mask, in_proj_w, in_proj_b, out_w, out_b,
           _trace=False):
    query = np.asarray(query, np.float32)
    key = np.asarray(key, np.float32)
    value = np.asarray(value, np.float32)
    domain_mask = np.asarray(domain_mask, np.float32)
    in_proj_w = np.asarray(in_proj_w, np.float32)
    in_proj_b = np.asarray(in_proj_b, np.float32)
    out_w = np.asarray(out_w, np.float32)
    out_b = np.asarray(out_b, np.float32)

    wq, wk, wv = np.split(in_proj_w, 3, axis=0)
    bq, bk, bv = np.split(in_proj_b, 3, axis=0)

    if "nc" not in _CACHE:
        _CACHE["nc"], _CACHE["lo"] = build_program()
    nc, lo = _CACHE["nc"], _CACHE["lo"]

    in_maps = [
        _prep_core_inputs(c, query, key, value, domain_mask, wq, wk, wv,
                          bq, bk, bv, out_w, out_b, lo)
        for c in range(8)
    ]
    res = bass_utils.run_bass_kernel_spmd(nc, in_maps, core_ids=list(range(8)),
                                          trace=_trace)
    _CACHE["last_exec_time_ns"] = res.exec_time_ns

    attn_output = np.zeros((B, S, E), np.float32)
    for b in range(B):
        for c in range(4 * b, 4 * b + 4):
            attn_output[b] += res.results[c]["part_out"]
    wmask = np.empty((B, H, S, S), np.float32)
    for b in range(B):
        for hg in range(H):
            wmask[b, hg] = res.results[4 * b + hg // 4]["wm_t"][hg % 4].T.astype(np.float32)
    return attn_output, wmask
